# revision 115
# baseline (speedup 1.0000x reference)
"""GatedLTMMemory kernel for 8 Trainium2 NeuronCores.

Data-parallel over the 4096 flattened (B,N) tokens: 512 tokens per core.
Memory-slot tables and weights are replicated. Per-selected-slot projections
are replaced by projecting the slot tables once and running a masked
full-softmax over all S slots (exactly equivalent math).

v7, 97.2us (v3 baseline 132.5us). What carries the speedup:
  - every parameter-only tensor is precomputed on the host (same class of
    preprocessing as the weight transposes): normalized slot tables,
    kk = Wqp^T k_hat^T so the selection scores contract over QD=320 in 3
    chunks instead of D=512 in 4, kp = (k_hat Wk^T)^T, vp = v_hat Wv^T in
    bf16 with the softmax-denominator ones-column baked in,
    Wqf = (Wq Wqp / sqrt(DH))^T, and the entire output projector folded
    through Wo: Wfin = Wo^T (Wout diag(ln_g)), Gram matrix G = Wo^T Wo / D
    for the LN variance, colsum(Wo)/D for the LN mean, colsum(Wout'),
    bout' = bout + Wout ln_b. The on-device Wo/Wv/Wk/Wq projections and
    the whole oT stage disappear.
  - selection must reproduce the reference top-32 EXACTLY (one flipped
    slot costs ~1e-1 rel error; tf32-quality scores measure 0.24): scores
    are computed effectively fp64-exact as fp32 matmuls over kk_hi plus a
    1-cycle/row bf16 correction over kk_lo = fp64(kk) - fp32(kk). Score
    error ~3e-7 vs the 3.7e-6 minimum top-32 gap on this input set.
  - exact top-32 threshold per token: 4 rounds of DVE max8/match_replace
    per 128-token tile; tiles 0-2 run inside the scores window, tile 3
    weaves through attention half 0 as per-head hooks so it never blocks
    the AV matmuls. Mask build (is_ge) on Pool; mask transposes to
    [slot, token] layout on the DMA xbar (dma_start_transpose from the
    ACT hwdge queue, late enough not to contend with the bulk loads).
  - attention is software-pipelined: each AV quad is emitted seven groups
    behind its QK quad so PE always has queued work while ACT computes
    the exps (the steady-state pacer at ~1038ns/group) and DVE/Pool apply
    the masks. Mask-multiplies split DVE/Pool per head to balance load.
  - LayerNorm is applied through the output matmul: PSUM accumulates
    Wfin@ctx + wcol x (-mu) + bout' x sd, then one DVE multiply by the
    partition-broadcast rstd. rstd comes from a table-free Quake-style
    rsqrt (bit trick + one Newton step) on DVE/Pool, so Exp is the only
    table function in the kernel -> exactly one LoadActFuncSet, and the
    epilogue for token-half 0 hides completely inside attention half 1.
  - PE p-state: dead memset-matmul warmup sized to the DMA prefix keeps
    the clock ramp off the critical path; kk/query stream in column
    pieces so score group t0h0 starts after ~1MB of DMA.
"""

import ml_dtypes as _ml_dtypes
import numpy as np

import concourse.bacc as bacc
import concourse.mybir as mybir
import concourse.tile as tile
from concourse.bass import ds, ts
from concourse.bass_utils import run_bass_kernel_spmd

B, N, QD, D, S, H, K = 4, 1024, 320, 512, 1024, 8, 32
DH = D // H
EPS = 1e-5
P = 128
T = 512                       # tokens per core
HT = 256                      # tokens per epilogue half
NCORES = 8
NT = T // P                   # 4 token tiles
NS = S // P                   # 8 slot chunks
NC = 3                        # contraction chunks over padded QD (384)
QDP = 384                     # padded QD
NEG = -1e30
QD_TILES = [(0, 128), (128, 128), (256, 64)]
WARMUP = 70

f32 = mybir.dt.float32
f32r = mybir.dt.float32r
bf16 = mybir.dt.bfloat16
AF = mybir.ActivationFunctionType
OP = mybir.AluOpType

_CACHE: dict = {}


def _build_nc():
    nc = bacc.Bacc("TRN2", target_bir_lowering=False, debug=False)

    dr = {}

    def din(name, shape, dt_):
        dr[name] = nc.dram_tensor(name, shape, dt_, kind="ExternalInput")

    din("queryT", (QDP, T), f32)
    din("kk_hi", (QDP, S), f32)
    din("kk_lo", (QDP, S), bf16)
    din("Wqf", (QDP, D), f32r)
    din("kp", (D, S), f32r)
    din("vp", (S, H * (DH + 1)), bf16)
    din("G", (D, D), f32r)          # Wo^T Wo / D (symmetric)
    din("WfinT", (D, QD), f32r)     # Wo^T (Wout * ln_g)
    din("wb", (1, 2 * D), f32r)     # row: [wcol | bout'] (QD cols each)
    din("wocol", (D,), f32r)        # colsum(Wo)/D as a column
    out_dram = nc.dram_tensor("outT", (QDP, T), f32, kind="ExternalOutput")

    with tile.TileContext(nc) as tc:
        with (
            tc.tile_pool(name="const", bufs=1) as const,
            tc.tile_pool(name="main", bufs=1) as main,
            tc.tile_pool(name="scr2", bufs=2) as scr2,
            tc.tile_pool(name="psmm", bufs=2, space="PSUM") as psmm,
            tc.tile_pool(name="psq", bufs=2, space="PSUM") as psq,
            tc.tile_pool(name="psctx", bufs=2, space="PSUM") as psctx,
            nc.allow_low_precision(reason="validated f32r/bf16 paths"),
        ):
            # ---------- constants ----------
            wm = const.tile([P, P], bf16, tag="wm")
            nc.vector.memset(wm, 0.5)
            ones_col = const.tile([P, 1], f32, tag="ones_col")
            nc.vector.memset(ones_col, 1.0)
            # selA/selB rows for per-head-pair denominator broadcast
            halfsel = const.tile([1, 2 * P], f32, tag="halfsel")
            nc.vector.memset(halfsel, 0.0)
            nc.vector.memset(halfsel[0:1, 64:192], 1.0)
            halfsel_r = const.tile([1, 2 * P], f32r, tag="halfsel_r")
            nc.scalar.copy(halfsel_r[:], halfsel[:])
            # layout: [0:64]=0, [64:192]=1, [192:256]=0
            ones_row_r = halfsel_r[0:1, 64:192]  # [1,128] ones
            selA = halfsel_r[0:1, 128:256]       # ones x64, zeros x64
            selB = halfsel_r[0:1, 0:128]         # zeros x64, ones x64
            ones_col_r = const.tile([P, 1], f32r, tag="ones_col_r")
            nc.scalar.copy(ones_col_r[:], ones_col[:])


            # PE p-state warmup: dead matmuls on a memset tile (no DMA dep)
            # keep the tensor engine busy so the ramp-to-full-clock window
            # burns off exactly while the first score operands stream in.
            ps_warm = psmm.tile([P, P], f32, tag="mm", name="warm")
            for i in range(WARMUP):
                nc.tensor.matmul(
                    ps_warm, lhsT=wm, rhs=wm,
                    start=True, stop=True, skip_group_check=True,
                )

            # ---------- DMA loads (critical tensors first) ----------
            def load_wide(name, nchunk, inner, dt_, tag, split=None):
                t_ = main.tile([P, nchunk, inner], dt_, tag=tag, name=f"ld_{tag}")
                src = dr[name].ap().rearrange("(a p) s -> p a s", p=P)
                if split is None:
                    nc.sync.dma_start(t_[:], src)
                else:
                    # issue in column pieces so early consumers start sooner
                    for lo, sz in split:
                        nc.sync.dma_start(
                            t_[:, :, ds(lo, sz)], src[:, :, ds(lo, sz)]
                        )
                return t_

            # queryT: token-tile-0 columns first so score group t0h0 can
            # start after ~1MB of DMA instead of ~2.4MB.
            qryT = main.tile([P, NC, T], f32, tag="qry", name="ld_qry")
            src_q = dr["queryT"].ap().rearrange("(a p) s -> p a s", p=P)
            nc.sync.dma_start(qryT[:, :, 0:P], src_q[:, :, 0:P])
            kk_hi = main.tile([P, NC, S], f32, tag="kkhi", name="ld_kkhi")
            kk_lo = main.tile([P, NC, S], bf16, tag="kklo", name="ld_kklo")
            src_hi = dr["kk_hi"].ap().rearrange("(a p) s -> p a s", p=P)
            src_lo = dr["kk_lo"].ap().rearrange("(a p) s -> p a s", p=P)
            nc.sync.dma_start(kk_hi[:, :, 0:T], src_hi[:, :, 0:T])
            nc.sync.dma_start(kk_lo[:, :, 0:T], src_lo[:, :, 0:T])
            nc.sync.dma_start(kk_hi[:, :, T:S], src_hi[:, :, T:S])
            nc.sync.dma_start(kk_lo[:, :, T:S], src_lo[:, :, T:S])
            nc.sync.dma_start(qryT[:, :, P:T], src_q[:, :, P:T])
            wqf = load_wide("Wqf", NC, D, f32r, "wqf")        # [128, 3, 512]
            kp = load_wide("kp", 4, S, f32r, "kp")            # [128, 4, 1024]
            vp_t = main.tile([P, NS, H, DH + 1], bf16, tag="vp", name="ld_vp")
            nc.sync.dma_start(
                vp_t[:], dr["vp"].ap().rearrange("(a p) x -> p a x", p=P)
            )
            g_t = load_wide("G", 4, D, f32r, "g")
            wfin = load_wide("WfinT", 4, QD, f32r, "wfin")    # [128, 4, 320]
            wb_row = const.tile([1, 2, D], f32r, tag="wb")
            nc.sync.dma_start(wb_row[:], dr["wb"].ap().rearrange("o (a s) -> o a s", a=2))
            wocol_c = const.tile([P, 4], f32r, tag="wocol")
            nc.sync.dma_start(wocol_c[:], dr["wocol"].ap().rearrange("(a p) -> p a", p=P))


            # bf16 copy of the query feeds the lo-correction (piecewise so
            # tile 0 is ready right after its columns land); f32r copy feeds
            # qh much later.
            qryTb = main.tile([P, NC, T], bf16, tag="qryb", name="qryb")
            nc.gpsimd.tensor_copy(qryTb[:, :, 0:P], qryT[:, :, 0:P])
            nc.gpsimd.tensor_copy(qryTb[:, :, P:T], qryT[:, :, P:T])
            qryTr = main.tile([P, NC, T], f32r, tag="qryr", name="qryr")
            nc.gpsimd.tensor_copy(qryTr[:], qryT[:])

            # ---------- scores[t, s] = query @ kk (fp32 + f32r lo) ----------
            # transposed 0/1 masks land in mT [slot, chunk, token] via the
            # DMA xbar (ACT hwdge queue; bypasses the SP bulk loads).
            mT = main.tile([P, NS, T], bf16, tag="mT", name="mT")

            sc = [
                main.tile([P, S], f32, tag=f"sc{tt}", name=f"sc{tt}")
                for tt in range(NT)
            ]
            masks = [
                main.tile([P, S], bf16, tag=f"mk{tt}", name=f"mk{tt}")
                for tt in range(NT)
            ]
            works = [
                main.tile([P, S], f32, tag=f"wk{tt % 2}", name=f"wk{tt}")
                for tt in range(NT)
            ]
            mxs = {}

            def emit_score_group(tt, hf):
                col = ds(hf * T, T)
                ps = psmm.tile([P, T], f32, tag="mm")
                for c in range(NC):
                    nc.tensor.matmul(
                        ps, lhsT=qryT[:, c, ts(tt, P)], rhs=kk_hi[:, c, col],
                        start=(c == 0), stop=False,
                    )
                for c in range(NC):
                    nc.tensor.matmul(
                        ps, lhsT=qryTb[:, c, ts(tt, P)], rhs=kk_lo[:, c, col],
                        start=False, stop=(c == NC - 1),
                    )
                nc.scalar.copy(sc[tt][:, col], ps)

            def topk_piece(tt, r):
                # round r of the top-32 extraction for tile tt (DVE), plus
                # mask build + xbar transpose on the final round.
                t_, m_, work = sc[tt], masks[tt], works[tt]
                cur = t_ if r == 0 else work
                mx = main.tile([P, 8], f32, tag=f"mx{tt}_{r}", name=f"mx{tt}_{r}")
                nc.vector.max(out=mx[:], in_=cur[:])
                if r < 3:
                    nc.vector.match_replace(
                        out=work[:], in_to_replace=mx[:], in_values=cur[:],
                        imm_value=NEG,
                    )
                else:
                    nc.gpsimd.tensor_scalar(
                        m_[:], t_[:], mx[:, 7:8], None, op0=OP.is_ge
                    )
                    nc.scalar.dma_start_transpose(mT[:, :, ts(tt, P)], m_[:])

            # tiles 0/1 top-k inline; tile 2's first two rounds fill the idle
            # DVE at the end of the scores window (its mask DMA-transpose
            # stays late so it doesn't contend with the bulk loads); the
            # rest weaves through attention half 0 as hooks so h0's
            # mask-multiply isn't queued behind it on DVE.
            for tt in (0, 1):
                emit_score_group(tt, 0)
                emit_score_group(tt, 1)
                for r in range(4):
                    topk_piece(tt, r)
            emit_score_group(2, 0)
            emit_score_group(2, 1)
            topk_piece(2, 0)
            topk_piece(2, 1)
            emit_score_group(3, 0)
            emit_score_group(3, 1)
            topk23 = [
                (lambda tt=tt, r=r: topk_piece(tt, r))
                for tt, r in [(2, 2), (2, 3), (3, 0), (3, 1), (3, 2), (3, 3)]
            ] + [None] * 2

            # ---------- qhT[e, t] = Wqf @ query  (f32r, /8 folded) ----------
            qh = []
            for e in range(4):
                t_ = main.tile([P, T], f32r, tag=f"qh{e}", name=f"qh{e}")
                ps = psmm.tile([P, T], f32, tag="mm")
                for c in range(NC):
                    nc.tensor.matmul(
                        ps, lhsT=wqf[:, c, ts(e, P)], rhs=qryTr[:, c, :],
                        start=(c == 0), stop=(c == NC - 1),
                    )
                nc.scalar.copy(t_[:], ps)
                qh.append(t_)

            # ---------- attention: per 256-token half, quads of 4 chunks -----
            ctxT_big = main.tile([P, 4, T], f32, tag="cx", name="cx")
            ctxT = [ctxT_big[:, dt_i, :] for dt_i in range(4)]


            def attention_half(half, hooks=None):
                # Software-pipelined: each AV quad is emitted two groups
                # behind its QK quad, so PE always has QK work in the queue
                # while ACT/DVE produce the masked exp weights. hooks: per-
                # head callables, emitted after the head's den chain.
                tok = ds(half * HT, HT)
                pool_heads = (1, 3, 5, 7) if half == 0 else (3,)
                state = {}
                pending = []

                def emit_qk(h, g):
                    et, ro = h // 2, (h % 2) * 64
                    if h % 2 == 0 and g == 0:
                        state[h] = (
                            scr2.tile([1, 2 * HT], f32r, tag="den",
                                      name=f"den{half}_{h}"),
                            psctx.tile([DH + 1, 2, HT], f32, tag="ctx",
                                       name=f"ctx{half}_{h}"),
                        )
                    ps_att = psq.tile([P, 4, HT], f32, tag="q")
                    for i in range(4):
                        nc.tensor.matmul(
                            ps_att[:, i, :],
                            lhsT=kp[:, et, :][ro : ro + DH, ts(4 * g + i, P)],
                            rhs=qh[et][ro : ro + DH, tok],
                            start=True, stop=True, skip_group_check=True,
                        )
                    w = main.tile(
                        [P, 4, HT], bf16, tag=f"w{(2 * h + g) % 8}",
                        name=f"w{half}_{h}_{g}",
                    )
                    nc.scalar.activation(w[:], ps_att, AF.Exp)
                    m_eng = nc.gpsimd if h in pool_heads else nc.vector
                    m_eng.tensor_tensor(
                        w[:], w[:], mT[:, 4 * g : 4 * g + 4, tok], OP.mult
                    )
                    return w

                def emit_av(h, g, w):
                    et, ro = h // 2, (h % 2) * 64
                    den_pair, ps_ctx2 = state[h - h % 2]
                    ps_ctx = ps_ctx2[:, h % 2, :]
                    for i in range(4):
                        nc.tensor.matmul(
                            ps_ctx, lhsT=vp_t[:, 4 * g + i, h, :],
                            rhs=w[:, i, :],
                            start=(g == 0 and i == 0), stop=(g == 1 and i == 3),
                            skip_group_check=True,
                        )
                    if g == 1:
                        if half == 0:
                            nc.scalar.copy(
                                ctxT[et][ro : ro + DH, tok].bitcast(f32r),
                                ps_ctx[0:DH, :],
                            )
                        else:
                            nc.vector.tensor_copy(
                                ctxT[et][ro : ro + DH, tok].bitcast(f32r),
                                ps_ctx[0:DH, :],
                            )
                    if g == 1 and h % 2 == 1:
                        nc.vector.reciprocal(
                            den_pair[0:1, :], ps_ctx2[DH : DH + 1, :, :]
                        )
                        ps_rb = psmm.tile([P, HT], f32, tag="mm")
                        nc.tensor.matmul(
                            ps_rb, lhsT=selA, rhs=den_pair[0:1, 0:HT],
                            start=True, stop=False,
                        )
                        nc.tensor.matmul(
                            ps_rb, lhsT=selB, rhs=den_pair[0:1, HT : 2 * HT],
                            start=False, stop=True,
                        )
                        nc.vector.tensor_tensor(
                            ctxT[et][:, tok].bitcast(f32r), ctxT[et][:, tok],
                            ps_rb, OP.mult,
                        )
                        if hooks is not None and hooks[h - 1] is not None:
                            hooks[h - 1]()
                        if hooks is not None and hooks[h] is not None:
                            hooks[h]()

                for h in range(H):
                    for g in range(2):
                        w = emit_qk(h, g)
                        pending.append((h, g, w))
                        if len(pending) > 7:
                            emit_av(*pending.pop(0))
                for item in pending:
                    emit_av(*item)

            def epilogue_parts(half):
                tok = ds(half * HT, HT)
                st = {}

                def part_mu(dcs):
                    # LN mean: mu = (colsum(Wo)/D) @ ctx, rank-1 per chunk;
                    # hookable so half 1 accumulates it inside attention.
                    if "ps_mu" not in st:
                        st["ps_mu"] = psctx.tile(
                            [P, T], f32, tag="ctx", name=f"ps_mu{half}"
                        )
                    for dc in dcs:
                        nc.tensor.matmul(
                            st["ps_mu"][0:1, 0:HT], lhsT=wocol_c[:, dc : dc + 1],
                            rhs=ctxT[dc][:, tok].bitcast(f32r),
                            start=(dc == 0), stop=False,
                            skip_group_check=True,
                        )

                def part_z():
                    # LN variance input straight from ctxT:
                    #   E[oT^2] = colsum(ctx o (G @ ctx)),  G = Wo^T Wo / D
                    # shares the psctx slot size ([128,512]f32 == 2KB/part)
                    if not st.get("mu_done"):
                        part_mu(range(4))
                    ps_mu = st["ps_mu"]
                    for e in range(4):
                        ps_z = psmm.tile([P, T], f32, tag="mm")
                        for dc in range(4):
                            nc.tensor.matmul(
                                ps_z[:, 0:HT], lhsT=g_t[:, dc, ts(e, P)],
                                rhs=ctxT[dc][:, tok].bitcast(f32r),
                                start=(dc == 0), stop=(dc == 3),
                            )
                        zq = scr2.tile([P, HT], f32r, tag=f"lnsq{e % 2}")
                        nc.vector.tensor_tensor(
                            zq[:], ctxT[e][:, tok], ps_z[:, 0:HT], OP.mult
                        )
                        nc.tensor.matmul(
                            ps_mu[0:1, HT : 2 * HT], lhsT=ones_col_r[:],
                            rhs=zq[:],
                            start=False, stop=(e == 3),
                            skip_group_check=True,
                        )

                def mu_hook_a():
                    part_mu([0, 1, 2])

                def mu_hook_b():
                    part_mu([3])
                    st["mu_done"] = True

                st["mu_hooks"] = (mu_hook_a, mu_hook_b)

                def part_c():
                    # half 0 runs inside attention half 1 where DVE is hot:
                    # put its serial chain on the idle Pool engine instead.
                    v = nc.gpsimd if half == 0 else nc.vector
                    ps_mu = st["ps_mu"]
                    i32 = mybir.dt.int32
                    mu_row = main.tile([1, HT], f32, tag="mu", name=f"mu{half}")
                    nc.scalar.copy(mu_row[:], ps_mu[0:1, 0:HT])
                    nmu_row = main.tile([1, HT], f32r, tag="nmu", name=f"nmu{half}")
                    nc.scalar.mul(nmu_row[:], mu_row[:], -1.0)
                    st["nmu"] = nmu_row
                    musq = main.tile([1, HT], f32, tag="musq", name=f"musq{half}")
                    v.tensor_tensor(musq[:], mu_row[:], mu_row[:], OP.mult)
                    var_row = main.tile([1, HT], f32, tag="var", name=f"var{half}")
                    nc.vector.scalar_tensor_tensor(
                        var_row[:], ps_mu[0:1, HT : 2 * HT], EPS, musq[:],
                        op0=OP.add, op1=OP.subtract,
                    )
                    # rsqrt via the bit trick + 2 Newton steps, no ACT table:
                    # keeps the whole kernel on one ACT function set (Exp),
                    # so no mid-kernel 1283ns table reloads.
                    y = main.tile([1, HT], f32, tag="qy", name=f"qy{half}")
                    nc.vector.tensor_scalar(
                        y[:].bitcast(i32), var_row[:].bitcast(i32), 1, None,
                        op0=OP.logical_shift_right,
                    )
                    nc.vector.tensor_scalar(
                        y[:].bitcast(i32), y[:].bitcast(i32), -1, 0x5F3759DF,
                        op0=OP.mult, op1=OP.add,
                    )
                    t_row = main.tile([1, HT], f32, tag="qt", name=f"qt{half}")
                    rstd_row = main.tile([1, HT], f32, tag="rstd", name=f"rstd{half}")
                    nsteps = 1
                    for step in range(nsteps):
                        v.tensor_tensor(t_row[:], var_row[:], y[:], OP.mult)
                        v.tensor_tensor(t_row[:], t_row[:], y[:], OP.mult)
                        nc.vector.tensor_scalar(
                            t_row[:], t_row[:], -0.5, 1.5, op0=OP.mult, op1=OP.add
                        )
                        out_ap = y[:] if step < nsteps - 1 else rstd_row[:]
                        v.tensor_tensor(out_ap, y[:], t_row[:], OP.mult)
                    sd_row = main.tile([1, HT], f32r, tag="sd", name=f"sd{half}")
                    v.tensor_tensor(sd_row[:], var_row[:], rstd_row[:], OP.mult)
                    rstdB = main.tile([P, HT], f32, tag=f"rstdB{half}", name=f"rstdB{half}")
                    nc.gpsimd.partition_broadcast(rstdB[:], rstd_row[:])
                    st.update(rstdB=rstdB, sd=sd_row)

                def part_d():
                    # out = (Wfin@ctx + wcol x (-mu) + bout' x sd) * rstdB
                    ot_sb = scr2.tile([P, 3, HT], f32, tag="ot")
                    nc.vector.memset(ot_sb[64:P, 2, :], 0.0)  # pad rows
                    for qt, (off, sz) in enumerate(QD_TILES):
                        ps = psmm.tile([P, T], f32, tag="mm")
                        for e in range(4):
                            nc.tensor.matmul(
                                ps[:sz, 0:HT], lhsT=wfin[:, e, ds(off, sz)],
                                rhs=ctxT[e][:, tok].bitcast(f32r),
                                start=(e == 0), stop=False,
                            )
                        nc.tensor.matmul(
                            ps[:sz, 0:HT], lhsT=wb_row[0:1, 0, ds(off, sz)],
                            rhs=st["nmu"][:], start=False, stop=False,
                        )
                        nc.tensor.matmul(
                            ps[:sz, 0:HT], lhsT=wb_row[0:1, 1, ds(off, sz)],
                            rhs=st["sd"][:], start=False, stop=True,
                        )
                        nc.vector.tensor_tensor(
                            ot_sb[:sz, qt, :], ps[:sz, 0:HT], st["rstdB"][:sz, :],
                            OP.mult,
                        )
                    for qt, (off, sz) in enumerate(QD_TILES):
                        dq = nc.sync if qt % 2 == 0 else nc.scalar
                        dq.dma_start(
                            out_dram.ap()[ds(off, sz), ds(half * HT, HT)],
                            ot_sb[:sz, qt, :],
                        )

                return [part_z, part_c, part_d, st["mu_hooks"]]

            attention_half(0, hooks=topk23)
            parts0 = epilogue_parts(0)
            parts1 = epilogue_parts(1)
            mu1a, mu1b = parts1[3]
            attention_half(1, hooks=[None, parts0[0], None, parts0[1],
                                     None, mu1a, None, mu1b])
            parts0[2]()
            parts1[0]()
            parts1[1]()
            parts1[2]()

    nc.compile()
    return nc


def _prep_in_maps(inputs):
    def c(a):
        return np.ascontiguousarray(a, dtype=np.float32)

    def c64(a):
        return np.asarray(a, dtype=np.float64)

    def l2n64(x):
        x = c64(x)
        return x / np.sqrt((x * x).sum(-1, keepdims=True) + 1e-12)

    q = np.asarray(inputs["query_states"], dtype=np.float32).reshape(B * N, QD)
    keys = l2n64(inputs["mem_keys"])        # [S, D] fp64
    vals = l2n64(inputs["mem_values"])

    # scores operand: kk = Wqp^T @ keys^T, split fp32-hi + residual-lo
    kk64 = c64(inputs["Wqp"]).T @ keys.T    # [QD, S]
    kk_hi = kk64.astype(np.float32)
    kk_lo = (kk64 - kk_hi).astype(np.float32)

    def padr(a, rows):
        out = np.zeros((rows, a.shape[1]), dtype=np.float32)
        out[: a.shape[0]] = a
        return out

    # attention operands (parameter-only, host-fused)
    wqf = (c64(inputs["Wq"]) @ c64(inputs["Wqp"]) / np.sqrt(DH)).T  # [QD, D]
    kp = (keys @ c64(inputs["Wk"]).T).T                             # [D, S]
    vph = (vals @ c64(inputs["Wv"]).T).reshape(S, H, DH)            # [S, H, DH]
    vp = np.ones((S, H, DH + 1), dtype=np.float32)
    vp[:, :, :DH] = vph
    # output projector: fold ln_g into Wout cols, ln_b+bout into bias;
    # fold Wo through everything (Wfin, Gram matrix for var, colsums)
    ln_g = c(inputs["ln_g"])
    ln_b = c(inputs["ln_b"])
    wo64 = c64(inputs["Wo"])
    wout2 = (c64(inputs["Wout"]) * c64(ln_g)[None, :]).T            # [D, QD]
    bout2 = c(inputs["bout"]) + c64(inputs["Wout"]).astype(np.float32) @ ln_b
    gmat = wo64.T @ wo64 / D                                        # [D, D]
    wfin = wo64.T @ wout2                                           # [D, QD]
    wb = np.zeros((1, 2 * D), dtype=np.float32)
    wb[0, :QD] = wout2.sum(axis=0)
    wb[0, D : D + QD] = bout2
    wocol = (wo64.sum(axis=0) / D).astype(np.float32)

    shared = {
        "kk_hi": padr(kk_hi, QDP),
        "kk_lo": padr(kk_lo, QDP).astype(_ml_dtypes.bfloat16),
        "Wqf": padr(c(wqf), QDP),
        "kp": c(kp),
        "vp": np.ascontiguousarray(
            vp.reshape(S, H * (DH + 1)), dtype=np.float32
        ).astype(_ml_dtypes.bfloat16),
        "G": c(gmat),
        "WfinT": c(wfin),
        "wb": wb,
        "wocol": wocol,

    }
    in_maps = []
    for core in range(NCORES):
        m = dict(shared)
        m["queryT"] = padr(c(q[core * T : (core + 1) * T, :].T), QDP)
        in_maps.append(m)
    return in_maps


def kernel(**inputs) -> np.ndarray:
    if "nc" not in _CACHE:
        _CACHE["nc"] = _build_nc()
    nc = _CACHE["nc"]
    in_maps = _prep_in_maps(inputs)
    res = run_bass_kernel_spmd(nc, in_maps, core_ids=list(range(NCORES)))
    out = np.empty((B * N, QD), dtype=np.float32)
    for core in range(NCORES):
        out[core * T : (core + 1) * T, :] = res.results[core]["outT"][:QD].T
    return out.reshape(B, N, QD)


# revision 116
# speedup vs baseline: 1.0278x; 1.0278x over previous
"""GatedLTMMemory kernel for 8 Trainium2 NeuronCores.

Data-parallel over the 4096 flattened (B,N) tokens: 512 tokens per core.
Memory-slot tables and weights are replicated. Per-selected-slot projections
are replaced by projecting the slot tables once and running a masked
full-softmax over all S slots (exactly equivalent math).

v7, 97.2us (v3 baseline 132.5us). What carries the speedup:
  - every parameter-only tensor is precomputed on the host (same class of
    preprocessing as the weight transposes): normalized slot tables,
    kk = Wqp^T k_hat^T so the selection scores contract over QD=320 in 3
    chunks instead of D=512 in 4, kp = (k_hat Wk^T)^T, vp = v_hat Wv^T in
    bf16 with the softmax-denominator ones-column baked in,
    Wqf = (Wq Wqp / sqrt(DH))^T, and the entire output projector folded
    through Wo: Wfin = Wo^T (Wout diag(ln_g)), Gram matrix G = Wo^T Wo / D
    for the LN variance, colsum(Wo)/D for the LN mean, colsum(Wout'),
    bout' = bout + Wout ln_b. The on-device Wo/Wv/Wk/Wq projections and
    the whole oT stage disappear.
  - selection must reproduce the reference top-32 EXACTLY (one flipped
    slot costs ~1e-1 rel error; tf32-quality scores measure 0.24): scores
    are computed effectively fp64-exact as fp32 matmuls over kk_hi plus a
    1-cycle/row bf16 correction over kk_lo = fp64(kk) - fp32(kk). Score
    error ~3e-7 vs the 3.7e-6 minimum top-32 gap on this input set.
  - exact top-32 threshold per token: 4 rounds of DVE max8/match_replace
    per 128-token tile; tiles 0-2 run inside the scores window, tile 3
    weaves through attention half 0 as per-head hooks so it never blocks
    the AV matmuls. Mask build (is_ge) on Pool; mask transposes to
    [slot, token] layout on the DMA xbar (dma_start_transpose from the
    ACT hwdge queue, late enough not to contend with the bulk loads).
  - attention is software-pipelined: each AV quad is emitted seven groups
    behind its QK quad so PE always has queued work while ACT computes
    the exps (the steady-state pacer at ~1038ns/group) and DVE/Pool apply
    the masks. Mask-multiplies split DVE/Pool per head to balance load.
  - LayerNorm is applied through the output matmul: PSUM accumulates
    Wfin@ctx + wcol x (-mu) + bout' x sd, then one DVE multiply by the
    partition-broadcast rstd. rstd comes from a table-free Quake-style
    rsqrt (bit trick + one Newton step) on DVE/Pool, so Exp is the only
    table function in the kernel -> exactly one LoadActFuncSet, and the
    epilogue for token-half 0 hides completely inside attention half 1.
  - PE p-state: dead memset-matmul warmup sized to the DMA prefix keeps
    the clock ramp off the critical path; kk/query stream in column
    pieces so score group t0h0 starts after ~1MB of DMA.
"""

import ml_dtypes as _ml_dtypes
import numpy as np

import concourse.bacc as bacc
import concourse.mybir as mybir
import concourse.tile as tile
from concourse.bass import ds, ts
from concourse.bass_utils import run_bass_kernel_spmd

B, N, QD, D, S, H, K = 4, 1024, 320, 512, 1024, 8, 32
DH = D // H
EPS = 1e-5
P = 128
T = 512                       # tokens per core
HT = 256                      # tokens per epilogue half
NCORES = 8
NT = T // P                   # 4 token tiles
NS = S // P                   # 8 slot chunks
NC = 3                        # contraction chunks over padded QD (384)
QDP = 384                     # padded QD
NEG = -1e30
QD_TILES = [(0, 128), (128, 128), (256, 64)]
WARMUP = 70

f32 = mybir.dt.float32
f32r = mybir.dt.float32r
bf16 = mybir.dt.bfloat16
AF = mybir.ActivationFunctionType
OP = mybir.AluOpType

_CACHE: dict = {}


def _build_nc():
    nc = bacc.Bacc("TRN2", target_bir_lowering=False, debug=False)

    dr = {}

    def din(name, shape, dt_):
        dr[name] = nc.dram_tensor(name, shape, dt_, kind="ExternalInput")

    din("queryT", (QDP, T), f32)
    din("kk_hi", (QDP, S), f32)
    din("kk_lo", (QDP, S), bf16)
    din("Wqf", (QDP, D), f32r)
    din("kp", (D, S), f32r)
    din("vp", (S, H * (DH + 1)), bf16)
    din("G", (D, D), f32r)          # Wo^T Wo / D (symmetric)
    din("WfinT", (D, QD), f32r)     # Wo^T (Wout * ln_g)
    din("wb", (1, 2 * D), f32r)     # row: [wcol | bout'] (QD cols each)
    din("wocol", (D,), f32r)        # colsum(Wo)/D as a column
    out_dram = nc.dram_tensor("outT", (QDP, T), f32, kind="ExternalOutput")

    with tile.TileContext(nc) as tc:
        with (
            tc.tile_pool(name="const", bufs=1) as const,
            tc.tile_pool(name="main", bufs=1) as main,
            tc.tile_pool(name="scr2", bufs=2) as scr2,
            tc.tile_pool(name="psmm", bufs=2, space="PSUM") as psmm,
            tc.tile_pool(name="psq", bufs=2, space="PSUM") as psq,
            tc.tile_pool(name="psctx", bufs=2, space="PSUM") as psctx,
            nc.allow_low_precision(reason="validated f32r/bf16 paths"),
        ):
            # ---------- constants ----------
            wm = const.tile([P, P], bf16, tag="wm")
            nc.vector.memset(wm, 0.5)
            ones_col = const.tile([P, 1], f32, tag="ones_col")
            nc.vector.memset(ones_col, 1.0)
            # selA/selB rows for per-head-pair denominator broadcast
            halfsel = const.tile([1, 2 * P], f32, tag="halfsel")
            nc.vector.memset(halfsel, 0.0)
            nc.vector.memset(halfsel[0:1, 64:192], 1.0)
            halfsel_r = const.tile([1, 2 * P], f32r, tag="halfsel_r")
            nc.scalar.copy(halfsel_r[:], halfsel[:])
            # layout: [0:64]=0, [64:192]=1, [192:256]=0
            ones_row_r = halfsel_r[0:1, 64:192]  # [1,128] ones
            selA = halfsel_r[0:1, 128:256]       # ones x64, zeros x64
            selB = halfsel_r[0:1, 0:128]         # zeros x64, ones x64
            ones_col_r = const.tile([P, 1], f32r, tag="ones_col_r")
            nc.scalar.copy(ones_col_r[:], ones_col[:])


            # PE p-state warmup: dead matmuls on a memset tile (no DMA dep)
            # keep the tensor engine busy so the ramp-to-full-clock window
            # burns off exactly while the first score operands stream in.
            ps_warm = psmm.tile([P, P], f32, tag="mm", name="warm")
            for i in range(WARMUP):
                nc.tensor.matmul(
                    ps_warm, lhsT=wm, rhs=wm,
                    start=True, stop=True, skip_group_check=True,
                )

            # ---------- DMA loads (critical tensors first) ----------
            def load_wide(name, nchunk, inner, dt_, tag, split=None):
                t_ = main.tile([P, nchunk, inner], dt_, tag=tag, name=f"ld_{tag}")
                src = dr[name].ap().rearrange("(a p) s -> p a s", p=P)
                if split is None:
                    nc.sync.dma_start(t_[:], src)
                else:
                    # issue in column pieces so early consumers start sooner
                    for lo, sz in split:
                        nc.sync.dma_start(
                            t_[:, :, ds(lo, sz)], src[:, :, ds(lo, sz)]
                        )
                return t_

            # queryT: token-tile-0 columns first so score group t0h0 can
            # start after ~1MB of DMA instead of ~2.4MB.
            qryT = main.tile([P, NC, T], f32, tag="qry", name="ld_qry")
            src_q = dr["queryT"].ap().rearrange("(a p) s -> p a s", p=P)
            nc.sync.dma_start(qryT[:, :, 0:P], src_q[:, :, 0:P])
            kk_hi = main.tile([P, NC, S], f32, tag="kkhi", name="ld_kkhi")
            kk_lo = main.tile([P, NC, S], bf16, tag="kklo", name="ld_kklo")
            src_hi = dr["kk_hi"].ap().rearrange("(a p) s -> p a s", p=P)
            src_lo = dr["kk_lo"].ap().rearrange("(a p) s -> p a s", p=P)
            nc.sync.dma_start(kk_hi[:, :, 0:T], src_hi[:, :, 0:T])
            nc.sync.dma_start(kk_lo[:, :, 0:T], src_lo[:, :, 0:T])
            nc.sync.dma_start(kk_hi[:, :, T:S], src_hi[:, :, T:S])
            nc.sync.dma_start(kk_lo[:, :, T:S], src_lo[:, :, T:S])
            nc.sync.dma_start(qryT[:, :, P:T], src_q[:, :, P:T])
            wqf = load_wide("Wqf", NC, D, f32r, "wqf")        # [128, 3, 512]
            kp = load_wide("kp", 4, S, f32r, "kp")            # [128, 4, 1024]
            vp_t = main.tile([P, NS, H, DH + 1], bf16, tag="vp", name="ld_vp")
            nc.sync.dma_start(
                vp_t[:], dr["vp"].ap().rearrange("(a p) x -> p a x", p=P)
            )
            g_t = load_wide("G", 4, D, f32r, "g")
            wfin = load_wide("WfinT", 4, QD, f32r, "wfin")    # [128, 4, 320]
            wb_row = const.tile([1, 2, D], f32r, tag="wb")
            nc.sync.dma_start(wb_row[:], dr["wb"].ap().rearrange("o (a s) -> o a s", a=2))
            wocol_c = const.tile([P, 4], f32r, tag="wocol")
            nc.sync.dma_start(wocol_c[:], dr["wocol"].ap().rearrange("(a p) -> p a", p=P))


            # bf16 copy of the query feeds the lo-correction (piecewise so
            # tile 0 is ready right after its columns land); f32r copy feeds
            # qh much later.
            qryTb = main.tile([P, NC, T], bf16, tag="qryb", name="qryb")
            nc.gpsimd.tensor_copy(qryTb[:, :, 0:P], qryT[:, :, 0:P])
            nc.gpsimd.tensor_copy(qryTb[:, :, P:T], qryT[:, :, P:T])
            qryTr = main.tile([P, NC, T], f32r, tag="qryr", name="qryr")
            nc.gpsimd.tensor_copy(qryTr[:], qryT[:])

            # ---------- scores[t, s] = query @ kk (fp32 + f32r lo) ----------
            # transposed 0/1 masks land in mT [slot, chunk, token] via the
            # DMA xbar (ACT hwdge queue; bypasses the SP bulk loads).
            mT = main.tile([P, NS, T], bf16, tag="mT", name="mT")

            sc = [
                main.tile([P, S], f32, tag=f"sc{tt}", name=f"sc{tt}")
                for tt in range(NT)
            ]
            masks = [
                main.tile([P, S], bf16, tag=f"mk{tt}", name=f"mk{tt}")
                for tt in range(NT)
            ]
            works = [
                main.tile([P, S], f32, tag=f"wk{tt % 2}", name=f"wk{tt}")
                for tt in range(NT)
            ]
            mxs = {}

            def emit_score_group(tt, hf):
                col = ds(hf * T, T)
                ps = psmm.tile([P, T], f32, tag="mm")
                for c in range(NC):
                    nc.tensor.matmul(
                        ps, lhsT=qryT[:, c, ts(tt, P)], rhs=kk_hi[:, c, col],
                        start=(c == 0), stop=False,
                    )
                for c in range(NC):
                    nc.tensor.matmul(
                        ps, lhsT=qryTb[:, c, ts(tt, P)], rhs=kk_lo[:, c, col],
                        start=False, stop=(c == NC - 1),
                    )
                nc.scalar.copy(sc[tt][:, col], ps)

            def topk_piece(tt, r):
                # round r of the top-32 extraction for tile tt (DVE), plus
                # mask build + xbar transpose on the final round.
                t_, m_, work = sc[tt], masks[tt], works[tt]
                cur = t_ if r == 0 else work
                mx = main.tile([P, 8], f32, tag=f"mx{tt}_{r}", name=f"mx{tt}_{r}")
                nc.vector.max(out=mx[:], in_=cur[:])
                if r < 3:
                    nc.vector.match_replace(
                        out=work[:], in_to_replace=mx[:], in_values=cur[:],
                        imm_value=NEG,
                    )
                else:
                    nc.gpsimd.tensor_scalar(
                        m_[:], t_[:], mx[:, 7:8], None, op0=OP.is_ge
                    )
                    nc.scalar.dma_start_transpose(mT[:, :, ts(tt, P)], m_[:])

            # tiles 0/1 top-k inline; tile 2's first two rounds fill the idle
            # DVE at the end of the scores window (its mask DMA-transpose
            # stays late so it doesn't contend with the bulk loads); the
            # rest weaves through attention half 0 as hooks so h0's
            # mask-multiply isn't queued behind it on DVE.
            for tt in (0, 1):
                emit_score_group(tt, 0)
                emit_score_group(tt, 1)
                for r in range(4):
                    topk_piece(tt, r)
            emit_score_group(2, 0)
            emit_score_group(2, 1)
            topk_piece(2, 0)
            topk_piece(2, 1)
            emit_score_group(3, 0)
            emit_score_group(3, 1)
            topk23 = [
                (lambda tt=tt, r=r: topk_piece(tt, r))
                for tt, r in [(2, 2), (2, 3), (3, 0), (3, 1), (3, 2), (3, 3)]
            ] + [None] * 2

            # ---------- qhT[e, t] = Wqf @ query  (f32r, /8 folded) ----------
            qh = []
            for e in range(4):
                t_ = main.tile([P, T], f32r, tag=f"qh{e}", name=f"qh{e}")
                ps = psmm.tile([P, T], f32, tag="mm")
                for c in range(NC):
                    nc.tensor.matmul(
                        ps, lhsT=wqf[:, c, ts(e, P)], rhs=qryTr[:, c, :],
                        start=(c == 0), stop=(c == NC - 1),
                    )
                nc.scalar.copy(t_[:], ps)
                qh.append(t_)

            # ---------- attention: per 256-token half, quads of 4 chunks -----
            ctxT_big = main.tile([P, 4, T], f32, tag="cx", name="cx")
            ctxT = [ctxT_big[:, dt_i, :] for dt_i in range(4)]


            def attention_half(half, hooks=None):
                # Software-pipelined: each AV quad is emitted two groups
                # behind its QK quad, so PE always has QK work in the queue
                # while ACT/DVE produce the masked exp weights. hooks: per-
                # head callables, emitted after the head's den chain.
                tok = ds(half * HT, HT)
                pool_heads = (1, 3, 5, 7) if half == 0 else (3,)
                state = {}
                pending = []

                def emit_qk(h, g):
                    et, ro = h // 2, (h % 2) * 64
                    if h % 2 == 0 and g == 0:
                        state[h] = (
                            scr2.tile([1, 2 * HT], f32r, tag="den",
                                      name=f"den{half}_{h}"),
                            psctx.tile([DH + 1, 2, HT], f32, tag="ctx",
                                       name=f"ctx{half}_{h}"),
                        )
                    ps_att = psq.tile([P, 4, HT], f32, tag="q")
                    for i in range(4):
                        nc.tensor.matmul(
                            ps_att[:, i, :],
                            lhsT=kp[:, et, :][ro : ro + DH, ts(4 * g + i, P)],
                            rhs=qh[et][ro : ro + DH, tok],
                            start=True, stop=True, skip_group_check=True,
                        )
                    w = main.tile(
                        [P, 4, HT], bf16, tag=f"w{(2 * h + g) % 8}",
                        name=f"w{half}_{h}_{g}",
                    )
                    nc.scalar.activation(w[:], ps_att, AF.Exp)
                    m_eng = nc.gpsimd if h in pool_heads else nc.vector
                    m_eng.tensor_tensor(
                        w[:], w[:], mT[:, 4 * g : 4 * g + 4, tok], OP.mult
                    )
                    return w

                def emit_av(h, g, w):
                    et, ro = h // 2, (h % 2) * 64
                    den_pair, ps_ctx2 = state[h - h % 2]
                    ps_ctx = ps_ctx2[:, h % 2, :]
                    for i in range(4):
                        nc.tensor.matmul(
                            ps_ctx, lhsT=vp_t[:, 4 * g + i, h, :],
                            rhs=w[:, i, :],
                            start=(g == 0 and i == 0), stop=(g == 1 and i == 3),
                            skip_group_check=True,
                        )
                    if g == 1:
                        if half == 0:
                            nc.scalar.copy(
                                ctxT[et][ro : ro + DH, tok].bitcast(f32r),
                                ps_ctx[0:DH, :],
                            )
                        else:
                            nc.vector.tensor_copy(
                                ctxT[et][ro : ro + DH, tok].bitcast(f32r),
                                ps_ctx[0:DH, :],
                            )
                    if g == 1 and h % 2 == 1:
                        nc.vector.reciprocal(
                            den_pair[0:1, :], ps_ctx2[DH : DH + 1, :, :]
                        )
                        ps_rb = psmm.tile([P, HT], f32, tag="mm")
                        nc.tensor.matmul(
                            ps_rb, lhsT=selA, rhs=den_pair[0:1, 0:HT],
                            start=True, stop=False,
                        )
                        nc.tensor.matmul(
                            ps_rb, lhsT=selB, rhs=den_pair[0:1, HT : 2 * HT],
                            start=False, stop=True,
                        )
                        nc.vector.tensor_tensor(
                            ctxT[et][:, tok].bitcast(f32r), ctxT[et][:, tok],
                            ps_rb, OP.mult,
                        )
                        if hooks is not None and hooks[h - 1] is not None:
                            hooks[h - 1]()
                        if hooks is not None and hooks[h] is not None:
                            hooks[h]()

                for h in range(H):
                    for g in range(2):
                        w = emit_qk(h, g)
                        pending.append((h, g, w))
                        if len(pending) > 7:
                            emit_av(*pending.pop(0))
                for item in pending:
                    emit_av(*item)

            def epilogue_parts(half):
                tok = ds(half * HT, HT)
                st = {}

                def part_mu(dcs):
                    # LN mean: mu = (colsum(Wo)/D) @ ctx, rank-1 per chunk;
                    # hookable so half 1 accumulates it inside attention.
                    if "ps_mu" not in st:
                        st["ps_mu"] = psctx.tile(
                            [P, T], f32, tag="ctx", name=f"ps_mu{half}"
                        )
                    for dc in dcs:
                        nc.tensor.matmul(
                            st["ps_mu"][0:1, 0:HT], lhsT=wocol_c[:, dc : dc + 1],
                            rhs=ctxT[dc][:, tok].bitcast(f32r),
                            start=(dc == 0), stop=False,
                            skip_group_check=True,
                        )

                def part_z():
                    # LN variance input straight from ctxT:
                    #   E[oT^2] = colsum(ctx o (G @ ctx)),  G = Wo^T Wo / D
                    # shares the psctx slot size ([128,512]f32 == 2KB/part)
                    if not st.get("mu_done"):
                        part_mu(range(4))
                    ps_mu = st["ps_mu"]
                    for e in range(4):
                        ps_z = psmm.tile([P, T], f32, tag="mm")
                        for dc in range(4):
                            nc.tensor.matmul(
                                ps_z[:, 0:HT], lhsT=g_t[:, dc, ts(e, P)],
                                rhs=ctxT[dc][:, tok].bitcast(f32r),
                                start=(dc == 0), stop=(dc == 3),
                            )
                        zq = scr2.tile([P, HT], f32r, tag=f"lnsq{e % 2}")
                        nc.vector.tensor_tensor(
                            zq[:], ctxT[e][:, tok], ps_z[:, 0:HT], OP.mult
                        )
                        nc.tensor.matmul(
                            ps_mu[0:1, HT : 2 * HT], lhsT=ones_col_r[:],
                            rhs=zq[:],
                            start=False, stop=(e == 3),
                            skip_group_check=True,
                        )

                def mu_hook_a():
                    part_mu([0, 1, 2])

                def mu_hook_b():
                    part_mu([3])
                    st["mu_done"] = True

                st["mu_hooks"] = (mu_hook_a, mu_hook_b)

                def part_c():
                    # half 0 runs inside attention half 1 where DVE is hot:
                    # put its serial chain on the idle Pool engine instead.
                    v = nc.gpsimd if half == 0 else nc.vector
                    ps_mu = st["ps_mu"]
                    i32 = mybir.dt.int32
                    mu_row = main.tile([1, HT], f32, tag="mu", name=f"mu{half}")
                    nc.scalar.copy(mu_row[:], ps_mu[0:1, 0:HT])
                    nmu_row = main.tile([1, HT], f32r, tag="nmu", name=f"nmu{half}")
                    nc.scalar.mul(nmu_row[:], mu_row[:], -1.0)
                    st["nmu"] = nmu_row
                    musq = main.tile([1, HT], f32, tag="musq", name=f"musq{half}")
                    v.tensor_tensor(musq[:], mu_row[:], mu_row[:], OP.mult)
                    var_row = main.tile([1, HT], f32, tag="var", name=f"var{half}")
                    nc.vector.scalar_tensor_tensor(
                        var_row[:], ps_mu[0:1, HT : 2 * HT], EPS, musq[:],
                        op0=OP.add, op1=OP.subtract,
                    )
                    # rsqrt via the bit trick + 2 Newton steps, no ACT table:
                    # keeps the whole kernel on one ACT function set (Exp),
                    # so no mid-kernel 1283ns table reloads.
                    y = main.tile([1, HT], f32, tag="qy", name=f"qy{half}")
                    nc.vector.tensor_scalar(
                        y[:].bitcast(i32), var_row[:].bitcast(i32), 1, None,
                        op0=OP.logical_shift_right,
                    )
                    nc.vector.tensor_scalar(
                        y[:].bitcast(i32), y[:].bitcast(i32), -1, 0x5F3759DF,
                        op0=OP.mult, op1=OP.add,
                    )
                    t_row = main.tile([1, HT], f32, tag="qt", name=f"qt{half}")
                    rstd_row = main.tile([1, HT], f32, tag="rstd", name=f"rstd{half}")
                    nsteps = 1
                    for step in range(nsteps):
                        v.tensor_tensor(t_row[:], var_row[:], y[:], OP.mult)
                        v.tensor_tensor(t_row[:], t_row[:], y[:], OP.mult)
                        nc.vector.tensor_scalar(
                            t_row[:], t_row[:], -0.5, 1.5, op0=OP.mult, op1=OP.add
                        )
                        out_ap = y[:] if step < nsteps - 1 else rstd_row[:]
                        v.tensor_tensor(out_ap, y[:], t_row[:], OP.mult)
                    sd_row = main.tile([1, HT], f32r, tag="sd", name=f"sd{half}")
                    v.tensor_tensor(sd_row[:], var_row[:], rstd_row[:], OP.mult)
                    rstdB = main.tile([P, HT], f32, tag=f"rstdB{half}", name=f"rstdB{half}")
                    nc.gpsimd.partition_broadcast(rstdB[:], rstd_row[:])
                    st.update(rstdB=rstdB, sd=sd_row)

                def part_d():
                    # out = (Wfin@ctx + wcol x (-mu) + bout' x sd) * rstdB
                    ot_sb = scr2.tile([P, 3, HT], f32, tag="ot")
                    nc.vector.memset(ot_sb[64:P, 2, :], 0.0)  # pad rows
                    for qt, (off, sz) in enumerate(QD_TILES):
                        ps = psmm.tile([P, T], f32, tag="mm")
                        for e in range(4):
                            nc.tensor.matmul(
                                ps[:sz, 0:HT], lhsT=wfin[:, e, ds(off, sz)],
                                rhs=ctxT[e][:, tok].bitcast(f32r),
                                start=(e == 0), stop=False,
                            )
                        nc.tensor.matmul(
                            ps[:sz, 0:HT], lhsT=wb_row[0:1, 0, ds(off, sz)],
                            rhs=st["nmu"][:], start=False, stop=False,
                        )
                        nc.tensor.matmul(
                            ps[:sz, 0:HT], lhsT=wb_row[0:1, 1, ds(off, sz)],
                            rhs=st["sd"][:], start=False, stop=True,
                        )
                        nc.vector.tensor_tensor(
                            ot_sb[:sz, qt, :], ps[:sz, 0:HT], st["rstdB"][:sz, :],
                            OP.mult,
                        )
                    for qt, (off, sz) in enumerate(QD_TILES):
                        dq = nc.sync if qt % 2 == 0 else nc.scalar
                        dq.dma_start(
                            out_dram.ap()[ds(off, sz), ds(half * HT, HT)],
                            ot_sb[:sz, qt, :],
                        )

                return [part_z, part_c, part_d, st["mu_hooks"]]

            attention_half(0, hooks=topk23)
            parts0 = epilogue_parts(0)
            attention_half(1, hooks=[None, parts0[0], None, parts0[1],
                                     None, None, None, None])
            parts1 = epilogue_parts(1)
            parts0[2]()
            parts1[0]()
            parts1[1]()
            parts1[2]()

    nc.compile()
    return nc


def _prep_in_maps(inputs):
    def c(a):
        return np.ascontiguousarray(a, dtype=np.float32)

    def c64(a):
        return np.asarray(a, dtype=np.float64)

    def l2n64(x):
        x = c64(x)
        return x / np.sqrt((x * x).sum(-1, keepdims=True) + 1e-12)

    q = np.asarray(inputs["query_states"], dtype=np.float32).reshape(B * N, QD)
    keys = l2n64(inputs["mem_keys"])        # [S, D] fp64
    vals = l2n64(inputs["mem_values"])

    # scores operand: kk = Wqp^T @ keys^T, split fp32-hi + residual-lo
    kk64 = c64(inputs["Wqp"]).T @ keys.T    # [QD, S]
    kk_hi = kk64.astype(np.float32)
    kk_lo = (kk64 - kk_hi).astype(np.float32)

    def padr(a, rows):
        out = np.zeros((rows, a.shape[1]), dtype=np.float32)
        out[: a.shape[0]] = a
        return out

    # attention operands (parameter-only, host-fused)
    wqf = (c64(inputs["Wq"]) @ c64(inputs["Wqp"]) / np.sqrt(DH)).T  # [QD, D]
    kp = (keys @ c64(inputs["Wk"]).T).T                             # [D, S]
    vph = (vals @ c64(inputs["Wv"]).T).reshape(S, H, DH)            # [S, H, DH]
    vp = np.ones((S, H, DH + 1), dtype=np.float32)
    vp[:, :, :DH] = vph
    # output projector: fold ln_g into Wout cols, ln_b+bout into bias;
    # fold Wo through everything (Wfin, Gram matrix for var, colsums)
    ln_g = c(inputs["ln_g"])
    ln_b = c(inputs["ln_b"])
    wo64 = c64(inputs["Wo"])
    wout2 = (c64(inputs["Wout"]) * c64(ln_g)[None, :]).T            # [D, QD]
    bout2 = c(inputs["bout"]) + c64(inputs["Wout"]).astype(np.float32) @ ln_b
    gmat = wo64.T @ wo64 / D                                        # [D, D]
    wfin = wo64.T @ wout2                                           # [D, QD]
    wb = np.zeros((1, 2 * D), dtype=np.float32)
    wb[0, :QD] = wout2.sum(axis=0)
    wb[0, D : D + QD] = bout2
    wocol = (wo64.sum(axis=0) / D).astype(np.float32)

    shared = {
        "kk_hi": padr(kk_hi, QDP),
        "kk_lo": padr(kk_lo, QDP).astype(_ml_dtypes.bfloat16),
        "Wqf": padr(c(wqf), QDP),
        "kp": c(kp),
        "vp": np.ascontiguousarray(
            vp.reshape(S, H * (DH + 1)), dtype=np.float32
        ).astype(_ml_dtypes.bfloat16),
        "G": c(gmat),
        "WfinT": c(wfin),
        "wb": wb,
        "wocol": wocol,

    }
    in_maps = []
    for core in range(NCORES):
        m = dict(shared)
        m["queryT"] = padr(c(q[core * T : (core + 1) * T, :].T), QDP)
        in_maps.append(m)
    return in_maps


def kernel(**inputs) -> np.ndarray:
    if "nc" not in _CACHE:
        _CACHE["nc"] = _build_nc()
    nc = _CACHE["nc"]
    in_maps = _prep_in_maps(inputs)
    res = run_bass_kernel_spmd(nc, in_maps, core_ids=list(range(NCORES)))
    out = np.empty((B * N, QD), dtype=np.float32)
    for core in range(NCORES):
        out[core * T : (core + 1) * T, :] = res.results[core]["outT"][:QD].T
    return out.reshape(B, N, QD)
